# revision 54
# baseline (speedup 1.0000x reference)
# GCN (2-layer GCNConv + BatchNorm + ReLU + global mean pool) on 8 TRN2 NeuronCores.
#
# Math (reference):
#   deg[v]  = in-degree incl. self-loop;  dinv = deg^-1/2
#   layer(x, W, b): h = D^-1/2 (A+I) D^-1/2 (x W) + b
#   h1 = relu(batchnorm(layer1));  h2 = layer2(h1);  out = segment_mean(h2, batch)
#
# Strategy (v3):
#   * Edges partitioned by SRC owner ("push" form).  Core k owns nodes
#     [k*SL,(k+1)*SL); it gathers per-edge messages from a small LOCAL table
#     (xs = dinv*x, 1.6 MB local DRAM) and accumulates them per destination.
#     A ReduceScatter over a [128 x 25600] bf16 blob (output 0.8 MB/core)
#     replaces the pull-form table AllGather; each core ends with the full
#     aggregate for its own shard.  Self-loops are added after the RS.
#   * Destination rows are sorted; round r holds the r-th occurrence of each
#     row (dma_scatter_add races on duplicate rows within one instruction).
#     Rounds 0-1 (dense) scatter into an SBUF parity-split accumulator with
#     NARROW per-piece out slices (the cost of an SBUF scatter is its out-
#     slice footprint, not its index count).  Rounds >= 2 (sparse tails)
#     scatter-add directly onto the DRAM blob after the accumulator dump,
#     at per-index cost, as 256 B rows holding two node slots (G parity
#     selects the 64-element half).
#   * Pad slots gather a zero table row and scatter it into a row that holds
#     no real data (dense: a gap row inside the piece's range; tail: the
#     blob row of a padding node), so padding only ever adds 0.0.
#   * Layer 2 never materializes per-node features: out = P(A2 h1 W2)+b2 =
#     (q . h1) W2 + b2 with q[u,g] built on the host from INDEX data only.
#   * BN stats via the accumulated A^T[A|1] matmul; cross-core sums use
#     AllGather + on-chip tree sum (cheaper than AllReduce in the model).
#   * Node l of a shard lives at SBUF partition (l%16)*8 + (l//16)//50,
#     group (l//16)%50 -- the layout that makes the RS output reload a
#     single fully-contiguous DMA.
#
# Host-side preprocessing uses only index data (edge_index, batch): degrees,
# edge partitioning/rounds, the q pooling matrix.  Feature data is only ever
# permuted/padded on the host, never computed with.

import os

import numpy as np

N_NODES = 50000
N_EDGES = 800000
D = 64
NCORES = 8
NUM_GRAPHS = 64
BN_EPS = 1e-5
SPLIT = 32768  # int16 index limit


class Cfg:
    def __init__(self, n, sl):
        self.N = n                    # total nodes
        self.SL = sl                  # owned nodes per core (6250)
        self.SLP = 6400               # padded slice rows
        assert self.SLP % 256 == 0 and self.SLP >= sl
        self.NT = self.SLP // 128     # node tiles per slice (50)
        self.GP = self.SLP // 16      # rankslots per partition (400)
        self.HG = self.GP // 2        # own/peer slots (200)
        self.NG = NCORES * self.SLP   # padded global rows (51200)
        self.CAP = 7680               # max slots per DMA instruction
        self.DENSE_R = 2              # rounds scattered into SBUF
        # node l -> SBUF partition (l%16)*8 + (l//16)//NT? see P() below
        self.ZROW = self.ptn(sl) * self.NT + self.grp(sl)
        self.UPAD = 196               # blob row (mod HG) of a padding node
        # pieces: list of dicts (filled by prepare_inputs)
        #   {'kind':'S','n':slots,'s0':slot0,'s1':slot1}
        #   {'kind':'D','n':slots,'par':0/1}
        self.pieces = []

    def ptn(self, l):
        return (l % 16) * 8 + (l // 16) // 50

    def grp(self, l):
        return (l // 16) % 50


LAST_EXEC_TIME_NS = None
_NC_CACHE = {}
_LAST_IN_MAPS = None


def build(cfg):
    import concourse.mybir as mybir
    import concourse.mybir as mb
    import concourse.tile as tile
    from concourse import bacc
    from concourse.masks import make_identity

    f32 = mybir.dt.float32
    bf16 = mybir.dt.bfloat16
    i16 = mybir.dt.int16
    SLP, NT, HG = cfg.SLP, cfg.NT, cfg.HG
    NN = float(cfg.N)
    RG = [list(range(NCORES))]
    pieces = cfg.pieces
    R = len(pieces)
    SMAX = max(p["n"] for p in pieces)
    tot_s = sum(p["n"] for p in pieces)

    nc = bacc.Bacc(
        "TRN2", target_bir_lowering=False, debug=False, num_devices=NCORES
    )

    # --- external inputs (per-core values supplied via in_maps) ---
    xsl = nc.declare_dram_parameter("xsl", [128, NT * D], f32, isOutput=False)
    qsl = nc.declare_dram_parameter("qsl", [128, NT * NUM_GRAPHS], f32, isOutput=False)
    dinv_in = nc.declare_dram_parameter("dinv_in", [128, NT], f32, isOutput=False)
    glo_d = nc.declare_dram_parameter("glo", [128, tot_s // 16], i16, isOutput=False)
    sct_d = nc.declare_dram_parameter("sct", [128, tot_s // 16], i16, isOutput=False)
    p1_d = nc.declare_dram_parameter("p1", [1, NUM_GRAPHS], f32, isOutput=False)
    w1_d = nc.declare_dram_parameter("w1", [D, D], f32, isOutput=False)
    b1_d = nc.declare_dram_parameter("b1", [D, 1], f32, isOutput=False)
    ga_d = nc.declare_dram_parameter("ga", [D, 1], f32, isOutput=False)
    be_d = nc.declare_dram_parameter("be", [D, 1], f32, isOutput=False)
    w2_d = nc.declare_dram_parameter("w2", [D, D], f32, isOutput=False)
    b2_d = nc.declare_dram_parameter("b2", [1, D], f32, isOutput=False)
    out_d = nc.declare_dram_parameter("out", [NUM_GRAPHS, D], f32, isOutput=True)

    # --- internal DRAM ---
    table = nc.dram_tensor("table", [128, NT * D], f32)
    rs_in = nc.dram_tensor("rs_in", [128, 2 * HG * D], bf16)
    rs_out = nc.dram_tensor("rs_out", [16, 2 * HG * D], bf16)
    sag_in = nc.dram_tensor("sag_in", [D, D + 1], f32)
    sag_out = nc.dram_tensor("sag_out", [NCORES * D, D + 1], f32, addr_space="Shared")
    oag_in = nc.dram_tensor("oag_in", [NUM_GRAPHS, D], f32)
    oag_out = nc.dram_tensor("oag_out", [NCORES * NUM_GRAPHS, D], f32,
                             addr_space="Shared")

    with tile.TileContext(nc) as tc:
        with (
            tc.tile_pool(name="const", bufs=1) as const,
            tc.tile_pool(name="persist", bufs=1) as persist,
            tc.tile_pool(name="work", bufs=3) as work,
            tc.tile_pool(name="idxp", bufs=2) as idxp,
            tc.tile_pool(name="msgp", bufs=2) as msgp,
            tc.tile_pool(name="msgbp", bufs=2) as msgbp,
            tc.tile_pool(name="spsum", bufs=1, space="PSUM") as spsum,
            tc.tile_pool(name="wpsum", bufs=3, space="PSUM") as wpsum,
        ):
            # --- early: x load, xs scale, table write; then acc zeroing ---
            own = persist.tile([128, HG, D], bf16, name="own")
            peer = persist.tile([128, HG, D], bf16, name="peer")
            dinvs = const.tile([128, NT], f32)
            nc.sync.dma_start(out=dinvs[:], in_=dinv_in[:, :])
            dinv_b = dinvs[:, :].rearrange("p (g o) -> p g o", o=1).to_broadcast(
                [128, NT, D]
            )
            xs = persist.tile([128, NT, D], f32, name="xs")
            table_v = table[:, :].rearrange("p (g d) -> p g d", d=D)
            xsl_v = xsl[:, :].rearrange("p (g d) -> p g d", d=D)
            xts = []
            for ci, g0 in enumerate(range(0, NT, 25)):
                eng = nc.sync if ci == 0 else nc.scalar
                xtc = work.tile([128, 25, D], f32, tag=f"xt{ci}", name=f"xt{ci}",
                                bufs=1)
                eng.dma_start(out=xtc[:], in_=xsl_v[:, g0 : g0 + 25, :])
                nc.vector.tensor_tensor(
                    out=xs[:, g0 : g0 + 25, :], in0=xtc[:],
                    in1=dinvs[:, g0 : g0 + 25].rearrange(
                        "p (g o) -> p g o", o=1).to_broadcast([128, 25, D]),
                    op=mybir.AluOpType.mult,
                )
                eng.dma_start(
                    out=table_v[:, g0 : g0 + 25, :], in_=xs[:, g0 : g0 + 25, :]
                )
                xts.append(xtc)
            # zero the accumulator in small slot chunks so the DVE scheduler
            # can interleave the critical xs ops and casts between them
            for h0 in range(0, HG, 25):
                nc.vector.memset(own[:, h0 : h0 + 25, :], 0.0)
                nc.vector.memset(peer[:, h0 : h0 + 25, :], 0.0)
            qs = persist.tile([128, NT, NUM_GRAPHS], f32, name="qs")

            # --- constants ---
            w1s = const.tile([D, D], f32)
            nc.sync.dma_start(out=w1s[:], in_=w1_d[:, :])
            w2s = const.tile([D, D], f32)
            nc.sync.dma_start(out=w2s[:], in_=w2_d[:, :])
            b1c = const.tile([D, 1], f32)
            nc.sync.dma_start(out=b1c[:], in_=b1_d[:, :])
            gac = const.tile([D, 1], f32)
            nc.sync.dma_start(out=gac[:], in_=ga_d[:, :])
            bec = const.tile([D, 1], f32)
            nc.sync.dma_start(out=bec[:], in_=be_d[:, :])
            b2r = const.tile([1, D], f32)
            nc.sync.dma_start(out=b2r[:], in_=b2_d[:, :])
            p1s = const.tile([1, NUM_GRAPHS], f32)
            nc.sync.dma_start(out=p1s[:], in_=p1_d[:, :])
            ident = const.tile([128, 128], f32)
            make_identity(nc, ident[:])
            ones64 = const.tile([D, 1], f32)
            nc.vector.memset(ones64[:], 1.0)
            epsc = const.tile([D, 1], f32)
            nc.vector.memset(epsc[:], BN_EPS)
            b1sq = persist.tile([D, 1], f32, name="b1sq")
            nc.vector.tensor_tensor(out=b1sq[:], in0=b1c[:], in1=b1c[:],
                                    op=mybir.AluOpType.mult)

            # b2 broadcast matrix (p1^T x b2); PE idle here
            b2_ps = wpsum.tile([NUM_GRAPHS, D], f32, tag="ps_b", name="b2_ps")
            nc.tensor.matmul(out=b2_ps[:], lhsT=p1s[:], rhs=b2r[:], start=True,
                             stop=True)
            b2m = persist.tile([NUM_GRAPHS, D], f32, name="b2m")
            nc.vector.tensor_copy(out=b2m[:], in_=b2_ps[:])

            # --- edge pipeline ---
            table_rows = table[:, :].rearrange("p (g d) -> (p g) d", d=D)
            # blob as 256 B rows: row u = p*HG + s, col half = G parity
            blob_rows = rs_in[:, :].rearrange("p (s c) -> (p s) c", c=2 * D)
            rs_in_v = rs_in[:, :].rearrange("p (r s d) -> p r s d", r=2, d=D)

            def edge_piece(c, pc):
                s = pc["n"]
                off = pc["off"]
                gidx = idxp.tile([128, SMAX // 16], i16, tag="gidx", name="gidx")
                nc.sync.dma_start(
                    out=gidx[:, : s // 16], in_=glo_d[:, off : off + s // 16]
                )
                sidx = idxp.tile([128, SMAX // 16], i16, tag="sidx", name="sidx")
                nc.sync.dma_start(
                    out=sidx[:, : s // 16], in_=sct_d[:, off : off + s // 16]
                )
                msg = msgp.tile([128, SMAX // 128, D], f32, tag="msg", name="msg")
                nc.gpsimd.dma_gather(
                    out_ap=msg[:, : s // 128, :],
                    in_ap=table_rows[0:SLP, :],
                    idxs_ap=gidx[:, : s // 16],
                    num_idxs=s, num_idxs_reg=s, elem_size=D,
                    single_packet=False, queue_num=0,
                )
                msgb = msgbp.tile([128, SMAX // 128, D], bf16, tag="msgb",
                                  name="msgb")
                # first casts on ACT while DVE drains the acc memsets
                if c >= 4 and c % 2 == 0:
                    nc.vector.tensor_copy(
                        out=msgb[:, : s // 128, :], in_=msg[:, : s // 128, :]
                    )
                else:
                    nc.scalar.activation(
                        msgb[:, : s // 128, :], msg[:, : s // 128, :],
                        mb.ActivationFunctionType.Copy,
                    )
                if pc["kind"] == "S":
                    s0, s1 = pc["s0"], pc["s1"]
                    nc.gpsimd.dma_scatter_add(
                        own[:, s0:s1, :], msgb[:, : s // 128, :],
                        sidx[:, : s // 16], s, s, D,
                        sbuf_tokens_per_rank=128, parity_reg=0,
                        out_ap_other=peer[:, s0:s1, :],
                        single_packet=False, queue_num=0,
                    )
                else:
                    # one gather bin feeding several blob scatter segments
                    for par, so, sn in pc["segs"]:
                        half = par * D
                        nc.gpsimd.dma_scatter_add(
                            blob_rows[:, half : half + D],
                            msgb[:, so // 128 : (so + sn) // 128, :],
                            sidx[:, so // 16 : (so + sn) // 16], sn, sn, D,
                            elem_step=2 * D,
                            single_packet=False, queue_num=0,
                        )

            for c in range(R):
                if pieces[c]["kind"] == "S":
                    edge_piece(c, pieces[c])

            # --- dump acc -> blob in slot-range quarters (each quarter can
            # start as soon as the last dense scatter touching it is done),
            # then sparse tails accumulate onto it ---
            for si, h0 in enumerate(range(0, HG, 50)):
                eng = nc.sync if si % 2 == 0 else nc.scalar
                eng.dma_start(out=rs_in_v[:, 0, h0 : h0 + 50, :],
                              in_=own[:, h0 : h0 + 50, :])
                eng.dma_start(out=rs_in_v[:, 1, h0 : h0 + 50, :],
                              in_=peer[:, h0 : h0 + 50, :])
            nc.sync.dma_start(
                out=qs[:], in_=qsl[:, :].rearrange("p (g d) -> p g d", d=NUM_GRAPHS)
            )
            for c in range(R):
                if pieces[c]["kind"] == "G":
                    edge_piece(c, pieces[c])

            # --- ReduceScatter + contiguous reload of the own shard ---
            nc.gpsimd.collective_compute(
                "ReduceScatter", mybir.AluOpType.add, replica_groups=RG,
                ins=[rs_in[:, :]], outs=[rs_out[:, :]],
            )
            agb = persist.tile([128, NT, D], bf16, name="agb")
            nc.sync.dma_start(
                out=agb[:].rearrange("p g d -> p (g d)"),
                in_=rs_out[:, :].rearrange("q (e c) -> (q e) c", e=8),
            )

            # --- aggs = (agb + xs) * dinv; BN stats; transposed h (pre-BN) ---
            aggs = persist.tile([128, NT, D + 1], f32, name="aggs")
            nc.vector.memset(aggs[:, :, D : D + 1], 1.0)
            stats_ps = spsum.tile([D, D + 1], f32, name="stats_ps")
            hT_big = persist.tile([D, NT * 128], bf16, name="hT_big")
            # pass 1: assemble aggs (gpsimd) + stats matmuls (PE) only, so the
            # stats AllGather can fire early and overlap the hT pipeline below
            for b0 in range(0, NT, 4):
                bn = min(4, NT - b0)
                nc.gpsimd.tensor_tensor(
                    out=aggs[:, b0 : b0 + bn, :D],
                    in0=agb[:, b0 : b0 + bn, :], in1=xs[:, b0 : b0 + bn, :],
                    op=mybir.AluOpType.add,
                )
                nc.gpsimd.tensor_tensor(
                    out=aggs[:, b0 : b0 + bn, :D],
                    in0=aggs[:, b0 : b0 + bn, :D],
                    in1=dinvs[:, b0 : b0 + bn].rearrange(
                        "p (g o) -> p g o", o=1).to_broadcast([128, bn, D]),
                    op=mybir.AluOpType.mult,
                )
                for j in range(bn):
                    b = b0 + j
                    nc.tensor.matmul(
                        out=stats_ps[:], lhsT=aggs[:, b, :D], rhs=aggs[:, b, :],
                        start=(b == 0), stop=(b == NT - 1),
                    )
            stats_sb = persist.tile([D, D + 1], f32, name="stats_sb")
            nc.scalar.activation(stats_sb[:], stats_ps[:],
                                 mb.ActivationFunctionType.Copy)
            nc.sync.dma_start(out=sag_in[:, :], in_=stats_sb[:])
            nc.gpsimd.collective_compute(
                "AllGather", mybir.AluOpType.bypass, replica_groups=RG,
                ins=[sag_in[:, :]], outs=[sag_out[:, :]],
            )
            # pass 2: transposed h (pre-BN) while the AllGather is in flight
            for b0 in range(0, NT, 4):
                bn = min(4, NT - b0)
                tp_ps = wpsum.tile([D, 512], f32, tag="ps_a", name="tp_ps")
                for j in range(bn):
                    b = b0 + j
                    nc.tensor.transpose(
                        out=tp_ps[:, j * 128 : (j + 1) * 128],
                        in_=aggs[:, b, :D], identity=ident[:],
                    )
                aggsT = work.tile([D, 512], f32, tag="aggsT", name="aggsT", bufs=2)
                nc.vector.tensor_copy(out=aggsT[:, : bn * 128],
                                      in_=tp_ps[:, : bn * 128])
                hT_ps = wpsum.tile([D, 512], f32, tag="ps_b", name="hT_ps")
                nc.tensor.matmul(
                    out=hT_ps[:, : bn * 128], lhsT=w1s[:],
                    rhs=aggsT[:, : bn * 128], start=True, stop=True,
                )
                nc.scalar.activation(
                    hT_big[:, b0 * 128 : (b0 + bn) * 128],
                    hT_ps[:, : bn * 128], mb.ActivationFunctionType.Copy,
                )

            # --- stats tree-sum + BN scalar algebra ---
            st8 = persist.tile([D, NCORES, D + 1], f32, name="st8")
            nc.sync.dma_start(
                out=st8[:], in_=sag_out[:, :].rearrange("(r p) c -> p r c", p=D)
            )
            nc.vector.tensor_tensor(
                out=st8[:, 0:4, :], in0=st8[:, 0:4, :], in1=st8[:, 4:8, :],
                op=mybir.AluOpType.add,
            )
            nc.vector.tensor_tensor(
                out=st8[:, 0:2, :], in0=st8[:, 0:2, :], in1=st8[:, 2:4, :],
                op=mybir.AluOpType.add,
            )
            st = persist.tile([D, D + 1], f32, name="st")
            nc.vector.tensor_tensor(
                out=st[:], in0=st8[:, 0, :], in1=st8[:, 1, :],
                op=mybir.AluOpType.add,
            )

            q_ps = wpsum.tile([D, 1], f32, tag="ps_a", name="q_ps")
            nc.tensor.matmul(out=q_ps[:], lhsT=w1s[:], rhs=st[:, D : D + 1],
                             start=True, stop=True)
            mu = persist.tile([D, 1], f32, name="mu")
            nc.vector.tensor_scalar(
                out=mu[:], in0=q_ps[:], scalar1=1.0 / NN, scalar2=b1c[:],
                op0=mybir.AluOpType.mult, op1=mybir.AluOpType.add,
            )
            t1_ps = wpsum.tile([D, D], f32, tag="ps_b", name="t1_ps")
            nc.tensor.matmul(out=t1_ps[:], lhsT=st[:, :D], rhs=w1s[:],
                             start=True, stop=True)
            m_sb = work.tile([D, D], f32, tag="m_sb", name="m_sb")
            nc.vector.tensor_tensor(out=m_sb[:], in0=w1s[:], in1=t1_ps[:],
                                    op=mybir.AluOpType.mult)
            d_ps = wpsum.tile([D, 1], f32, tag="ps_b", name="d_ps")
            nc.tensor.matmul(out=d_ps[:], lhsT=m_sb[:], rhs=ones64[:],
                             start=True, stop=True)

            var = persist.tile([D, 1], f32, name="var")
            t2 = work.tile([D, 1], f32, tag="t2", name="t2")
            nc.vector.tensor_scalar(
                out=t2[:], in0=q_ps[:], scalar1=2.0 / NN, scalar2=b1c[:],
                op0=mybir.AluOpType.mult, op1=mybir.AluOpType.mult,
            )
            nc.vector.tensor_scalar(
                out=var[:], in0=d_ps[:], scalar1=1.0 / NN, scalar2=t2[:],
                op0=mybir.AluOpType.mult, op1=mybir.AluOpType.add,
            )
            nc.vector.tensor_tensor(out=var[:], in0=var[:], in1=b1sq[:],
                                    op=mybir.AluOpType.add)
            t4 = work.tile([D, 1], f32, tag="t4", name="t4")
            nc.vector.tensor_tensor(out=t4[:], in0=mu[:], in1=mu[:],
                                    op=mybir.AluOpType.mult)
            nc.vector.tensor_tensor(out=var[:], in0=var[:], in1=t4[:],
                                    op=mybir.AluOpType.subtract)

            sd = work.tile([D, 1], f32, tag="sd", name="sd")
            nc.scalar.activation(sd[:], var[:], mb.ActivationFunctionType.Sqrt,
                                 bias=epsc[:])
            rstd = work.tile([D, 1], f32, tag="rstd", name="rstd")
            nc.vector.reciprocal(out=rstd[:], in_=sd[:])
            a_sb = persist.tile([D, 1], f32, name="a_sb")
            nc.vector.tensor_tensor(out=a_sb[:], in0=gac[:], in1=rstd[:],
                                    op=mybir.AluOpType.mult)
            c_sb = persist.tile([D, 1], f32, name="c_sb")
            t5 = work.tile([D, 1], f32, tag="t5", name="t5")
            nc.vector.tensor_tensor(out=t5[:], in0=mu[:], in1=a_sb[:],
                                    op=mybir.AluOpType.mult)
            nc.vector.tensor_tensor(out=c_sb[:], in0=bec[:], in1=t5[:],
                                    op=mybir.AluOpType.subtract)
            # hT excludes the b1 bias; fold it into the BN offset:
            # relu(a*(h+b1) + c) = relu(a*h + (c + a*b1))
            t6 = work.tile([D, 1], f32, tag="t6", name="t6")
            nc.vector.tensor_tensor(out=t6[:], in0=a_sb[:], in1=b1c[:],
                                    op=mybir.AluOpType.mult)
            nc.vector.tensor_tensor(out=c_sb[:], in0=c_sb[:], in1=t6[:],
                                    op=mybir.AluOpType.add)

            # --- BN+ReLU, transpose back, pool matmul ---
            h1 = persist.tile([128, NT, D], f32, name="h1")
            poolT_ps = spsum.tile([D, NUM_GRAPHS], f32, name="poolT_ps")
            for b0 in range(0, NT, 4):
                bn = min(4, NT - b0)
                h1T = work.tile([D, 512], f32, tag="h1T", name="h1T", bufs=2)
                nc.scalar.activation(
                    h1T[:, : bn * 128],
                    hT_big[:, b0 * 128 : (b0 + bn) * 128],
                    mb.ActivationFunctionType.Relu, bias=c_sb[:], scale=a_sb[:],
                )
                for j in range(bn):
                    b = b0 + j
                    nm_ps = wpsum.tile([128, D], f32, tag="ps_a", name="nm_ps")
                    nc.tensor.transpose(
                        out=nm_ps[:], in_=h1T[:, j * 128 : (j + 1) * 128],
                        identity=ident[:D, :D],
                    )
                    nc.vector.tensor_copy(out=h1[:, b, :], in_=nm_ps[:])
                    nc.tensor.matmul(
                        out=poolT_ps[:], lhsT=h1[:, b, :], rhs=qs[:, b, :],
                        start=(b == 0), stop=(b == NT - 1),
                    )

            # --- out partial, AllGather, sum, +b2, store ---
            poolT_sb = persist.tile([D, NUM_GRAPHS], f32, name="poolT_sb")
            nc.vector.tensor_copy(out=poolT_sb[:], in_=poolT_ps[:])
            out_ps = wpsum.tile([NUM_GRAPHS, D], f32, tag="ps_b", name="out_ps")
            nc.tensor.matmul(out=out_ps[:], lhsT=poolT_sb[:], rhs=w2s[:],
                             start=True, stop=True)
            out_sb = persist.tile([NUM_GRAPHS, D], f32, name="out_sb")
            nc.vector.tensor_copy(out=out_sb[:], in_=out_ps[:])
            nc.sync.dma_start(out=oag_in[:, :], in_=out_sb[:])
            nc.gpsimd.collective_compute(
                "AllGather", mybir.AluOpType.bypass, replica_groups=RG,
                ins=[oag_in[:, :]], outs=[oag_out[:, :]],
            )
            o8 = persist.tile([NUM_GRAPHS, NCORES, D], f32, name="o8")
            nc.sync.dma_start(
                out=o8[:],
                in_=oag_out[:, :].rearrange("(r p) c -> p r c", p=NUM_GRAPHS),
            )
            nc.vector.tensor_tensor(
                out=o8[:, 0:4, :], in0=o8[:, 0:4, :], in1=o8[:, 4:8, :],
                op=mybir.AluOpType.add,
            )
            nc.vector.tensor_tensor(
                out=o8[:, 0:2, :], in0=o8[:, 0:2, :], in1=o8[:, 2:4, :],
                op=mybir.AluOpType.add,
            )
            outf = persist.tile([NUM_GRAPHS, D], f32, name="outf")
            nc.vector.tensor_tensor(
                out=outf[:], in0=o8[:, 0, :], in1=o8[:, 1, :],
                op=mybir.AluOpType.add,
            )
            nc.vector.tensor_tensor(
                out=outf[:], in0=outf[:], in1=b2m[:], op=mybir.AluOpType.add,
            )
            nc.sync.dma_start(out=out_d[:, :], in_=outf[:])

    nc.compile()
    return nc


def _wrap16(v, n):
    """idx j at [j%16, j//16], replicated to 128 partitions (8 Q7 cores)."""
    assert v.shape[0] == n and n % 16 == 0
    t = v.astype(np.int16).reshape(n // 16, 16).T
    return np.tile(t, (8, 1))


def _pad_row(rows, limit):
    """Smallest value in [0, limit) absent from sorted unique `rows`."""
    miss = np.flatnonzero(rows != np.arange(len(rows)))
    cand = int(miss[0]) if len(miss) else len(rows)
    assert 0 <= cand < limit, (cand, limit)
    return cand


def _up128(v):
    return ((v + 127) // 128) * 128 if v else 0


def prepare_inputs(cfg, x, edge_index, batch, W1, b1, gamma, beta, W2, b2):
    """Host-side index preprocessing + per-core input maps. Fills cfg.pieces."""
    SL, SLP, NT, HG = cfg.SL, cfg.SLP, cfg.NT, cfg.HG
    n = cfg.N

    x = np.ascontiguousarray(np.asarray(x, dtype=np.float32))
    src = np.asarray(edge_index[0], dtype=np.int64)
    dst = np.asarray(edge_index[1], dtype=np.int64)
    batch = np.asarray(batch, dtype=np.int64)
    W1 = np.asarray(W1, dtype=np.float32)
    b1 = np.asarray(b1, dtype=np.float32)
    gamma = np.asarray(gamma, dtype=np.float32)
    beta = np.asarray(beta, dtype=np.float32)
    W2 = np.asarray(W2, dtype=np.float32)
    b2 = np.asarray(b2, dtype=np.float32)

    deg = np.bincount(dst, minlength=n).astype(np.float64) + 1.0  # + self-loop
    dinv = 1.0 / np.sqrt(deg)

    cnt = np.bincount(batch, minlength=NUM_GRAPHS).astype(np.float64)
    w_graph = 1.0 / np.maximum(cnt, 1.0)

    # q pooling matrix for layer 2 (index data only)
    wg = w_graph[batch]
    q = np.bincount(
        src * NUM_GRAPHS + batch[dst],
        weights=dinv[src] * dinv[dst] * wg[dst],
        minlength=n * NUM_GRAPHS,
    )
    q += np.bincount(
        np.arange(n) * NUM_GRAPHS + batch,
        weights=dinv * dinv * wg,
        minlength=n * NUM_GRAPHS,
    )
    q = q.reshape(n, NUM_GRAPHS).astype(np.float32)
    p1 = (cnt > 0).astype(np.float32).reshape(1, NUM_GRAPHS)
    dinv = dinv.astype(np.float32)

    # scatter row encoding: dst -> partition 16*owner + l%16, rankslot
    # 2*(G%HG) + G//HG with G = l//16;  sidx = rankslot*128 + partition
    o_d = dst // SL
    l_d = dst - o_d * SL
    G = l_d >> 4
    part_d = 16 * o_d + (l_d & 15)
    sidx = (2 * (G % HG) + (G // HG)) * 128 + part_d
    # blob row for the sparse-tail path: u = partition*HG + G//2, parity G%2
    udx = part_d * HG + (G >> 1)
    upar = G & 1
    # gather row (partition-major local table): u = P(l)*NT + (l//16)%50
    o_s = src // SL
    l_s = src - o_s * SL
    gidx = ((l_s % 16) * 8 + (l_s // 16) // 50) * NT + (l_s // 16) % 50

    DR = cfg.DENSE_R
    per_core = []  # [core] -> (dense rounds list, tail rounds list)
    ndense = 0
    ntail = 0
    for k in range(NCORES):
        sel = o_s == k
        sid = sidx[sel]
        gid = gidx[sel]
        uid = udx[sel]
        upr = upar[sel]
        order = np.argsort(sid, kind="stable")
        sid, gid, uid, upr = sid[order], gid[order], uid[order], upr[order]
        change = np.r_[True, sid[1:] != sid[:-1]] if len(sid) else np.zeros(0, bool)
        starts = np.flatnonzero(change)
        gg = np.cumsum(change) - 1
        occ = np.arange(len(sid)) - starts[gg] if len(sid) else np.zeros(0, np.int64)
        nr = int(occ.max()) + 1 if len(occ) else 0
        dense = []
        for r in range(min(nr, DR)):
            m = occ == r
            dense.append((sid[m], gid[m]))
        tails = []
        for r in range(DR, nr):
            m = occ == r
            for par in (0, 1):
                mp = m & (upr == par)
                # sort by blob row for uniqueness bookkeeping (optional)
                tails.append((r, par, uid[mp], gid[mp]))
        per_core.append((dense, tails))
        ndense = max(ndense, min(nr, DR))
        ntail = max(ntail, len(tails))

    # --- common piece structure ---
    cfg.pieces = []
    dense_meta = []   # (round, pos0, pos1) per piece
    for r in range(ndense):
        A = _up128(max(len(pc[0][r][0]) if r < len(pc[0]) else 0
                       for pc in per_core))
        pos = 0
        while pos < A:
            cap = cfg.CAP // 2 if (r == 0 and pos == 0) else cfg.CAP
            en = min(pos + cap, A)
            # slice range: min/max rankslot over cores in these positions
            rs_lo, rs_hi = 1 << 30, -1
            for pc in per_core:
                rows = pc[0][r][0] if r < len(pc[0]) else np.zeros(0, np.int64)
                seg = rows[pos:en]
                if len(seg):
                    rs_lo = min(rs_lo, int(seg[0]) >> 7)
                    rs_hi = max(rs_hi, int(seg[-1]) >> 7)
            if rs_hi < 0:
                rs_lo, rs_hi = 0, 0
            rs0 = rs_lo & ~1
            rs1 = ((rs_hi + 2) & ~1)
            assert rs1 - rs0 <= 256, (rs0, rs1)
            cfg.pieces.append(
                {"kind": "S", "n": en - pos, "s0": rs0 >> 1, "s1": rs1 >> 1}
            )
            dense_meta.append((r, pos, en))
            pos = en
    # tail segments (one per (round,parity), split at CAP), packed into
    # gather bins of <= CAP slots: one gather+cast feeds several scatters
    segs = []         # (t, pos, en)
    for t in range(ntail):
        A = _up128(max(len(pc[1][t][2]) if t < len(pc[1]) else 0
                       for pc in per_core))
        if A == 0:
            A = 128
        pos = 0
        while pos < A:
            en = min(pos + cfg.CAP, A)
            segs.append((t, pos, en))
            pos = en
    tpar = [max((pc[1][t][1] if t < len(pc[1]) else 0) for pc in per_core)
            for t in range(ntail)]
    tail_meta = []    # per 'G' piece: list of (t, pos, en) segments
    cur, cur_n = [], 0
    for t, pos, en in segs:
        if cur_n + (en - pos) > cfg.CAP and cur:
            cfg.pieces.append({"kind": "G", "n": cur_n, "segs": []})
            tail_meta.append(cur)
            cur, cur_n = [], 0
        cur.append((t, pos, en))
        cur_n += en - pos
    if cur:
        cfg.pieces.append({"kind": "G", "n": cur_n, "segs": []})
        tail_meta.append(cur)
    gi_piece = 0
    for pc in cfg.pieces:
        if pc["kind"] != "G":
            continue
        so = 0
        for t, pos, en in tail_meta[gi_piece]:
            pc["segs"].append((int(tpar[t]), so, en - pos))
            so += en - pos
        gi_piece += 1
    off = 0
    for pc in cfg.pieces:
        pc["off"] = off
        off += pc["n"] // 16

    # --- per-core index arrays ---
    in_maps = []
    for k in range(NCORES):
        dense, tails = per_core[k]
        glo_parts, sct_parts = [], []
        di = ti = 0
        for pc in cfg.pieces:
            npc = pc["n"]
            if pc["kind"] == "S":
                r, pos, en = dense_meta[di]; di += 1
                rows = dense[r][0] if r < len(dense) else np.zeros(0, np.int64)
                gids = dense[r][1] if r < len(dense) else np.zeros(0, np.int64)
                seg_r = rows[pos:en]
                seg_g = gids[pos:en]
                base = pc["s0"] * 256
                rel = seg_r - base
                assert len(rel) == 0 or (rel.min() >= 0 and
                                         rel.max() < (pc["s1"] - pc["s0"]) * 256)
                pad = _pad_row(np.unique(rel), (pc["s1"] - pc["s0"]) * 256)
                sarr = np.full(npc, pad, dtype=np.int64)
                sarr[: len(rel)] = rel
                garr = np.full(npc, cfg.ZROW, dtype=np.int64)
                garr[: len(seg_g)] = seg_g
            else:
                sarr = np.empty(npc, dtype=np.int64)
                garr = np.empty(npc, dtype=np.int64)
                so = 0
                for t, pos, en in tail_meta[ti]:
                    if t < len(tails):
                        _, _, uids, gids = tails[t]
                    else:
                        uids = np.zeros(0, np.int64)
                        gids = np.zeros(0, np.int64)
                    seg_u = uids[pos:en]
                    seg_g = gids[pos:en]
                    sn = en - pos
                    sseg = np.full(sn, cfg.UPAD, dtype=np.int64)
                    sseg[: len(seg_u)] = seg_u
                    gseg = np.full(sn, cfg.ZROW, dtype=np.int64)
                    gseg[: len(seg_g)] = seg_g
                    sarr[so : so + sn] = sseg
                    garr[so : so + sn] = gseg
                    so += sn
                assert so == npc
                ti += 1
            glo_parts.append(_wrap16(garr, npc))
            sct_parts.append(_wrap16(sarr, npc))
        glo = np.concatenate(glo_parts, axis=1)
        sct = np.concatenate(sct_parts, axis=1)

        lo, hi = k * SL, (k + 1) * SL
        nsl = hi - lo
        ll = np.arange(SLP)
        P_arr = (ll % 16) * 8 + (ll // 16) // 50
        G_arr = (ll // 16) % 50
        xflat = np.zeros((SLP, D), dtype=np.float32)
        xflat[:nsl] = x[lo:hi]
        xsl = np.zeros((128, NT, D), dtype=np.float32)
        xsl[P_arr, G_arr] = xflat
        xsl = xsl.reshape(128, NT * D)
        qflat = np.zeros((SLP, NUM_GRAPHS), dtype=np.float32)
        qflat[:nsl] = q[lo:hi]
        qsl = np.zeros((128, NT, NUM_GRAPHS), dtype=np.float32)
        qsl[P_arr, G_arr] = qflat
        qsl = qsl.reshape(128, NT * NUM_GRAPHS)
        dflat = np.zeros(SLP, dtype=np.float32)
        dflat[:nsl] = dinv[lo:hi]
        dinv_t = np.zeros((128, NT), dtype=np.float32)
        dinv_t[P_arr, G_arr] = dflat

        im = {
            "xsl": np.ascontiguousarray(xsl),
            "qsl": np.ascontiguousarray(qsl),
            "dinv_in": dinv_t,
            "glo": np.ascontiguousarray(glo),
            "sct": np.ascontiguousarray(sct),
            "p1": p1,
            "w1": W1,
            "b1": b1.reshape(D, 1),
            "ga": gamma.reshape(D, 1),
            "be": beta.reshape(D, 1),
            "w2": W2,
            "b2": b2.reshape(1, D),
        }
        in_maps.append(im)
    return in_maps


def kernel(x, edge_index, batch, W1, b1, gamma, beta, W2, b2):
    global LAST_EXEC_TIME_NS
    from concourse.bass_utils import run_bass_kernel_spmd

    cfg = Cfg(N_NODES, N_NODES // NCORES)
    in_maps = prepare_inputs(cfg, x, edge_index, batch, W1, b1, gamma, beta, W2, b2)

    key = (cfg.N, cfg.SL,
           tuple((p["kind"], p["n"], p.get("s0", 0), p.get("s1", 0),
                  p.get("par", 0)) for p in cfg.pieces))
    if key not in _NC_CACHE:
        _NC_CACHE[key] = build(cfg)
    nc = _NC_CACHE[key]
    global _LAST_IN_MAPS
    _LAST_IN_MAPS = in_maps

    trace = bool(int(os.environ.get("BASS_GNN_TRACE", "0")))
    if trace:
        try:
            res = run_bass_kernel_spmd(nc, in_maps, list(range(NCORES)), trace=True)
        except Exception:
            res = run_bass_kernel_spmd(nc, in_maps, list(range(NCORES)), trace=False)
    else:
        res = run_bass_kernel_spmd(nc, in_maps, list(range(NCORES)), trace=False)
    LAST_EXEC_TIME_NS = res.exec_time_ns
    return np.asarray(res.results[0]["out"], dtype=np.float32)


def modeled_time_ns(x=None, edge_index=None, **kw):
    """Cost-model execution time (MultiCoreSim, mocked collectives) for the
    current cached program; used when NTFF tracing is unavailable."""
    if not _NC_CACHE:
        return None
    nc = next(iter(_NC_CACHE.values()))
    ins = _LAST_IN_MAPS
    if ins is None:
        return None
    from concourse.bass_interp import MultiCoreSim

    sim = MultiCoreSim(nc, 2, debug_mock_collectives_without_correctness=True)
    for i, core in sim.cores.items():
        for name, val in ins[i].items():
            core.tensor(name)[:] = val
    sim.simulate()
    return int(sim.global_time)
